# revision 20
# baseline (speedup 1.0000x reference)
"""Trainium2 Bass kernel for nn_CPCAR_15960098472658 (ragged_sequence).

Pipeline (per batch element): variance-based segmentation (host, data
dependent) -> segment-mean compress (device matmul) -> GRU over compressed
sequence (device, serial scan) -> decompress by broadcasting each segment's
GRU output back to its frames (device DMA gather).

Sharding: data-parallel over batch. B=64 across 8 cores -> 8 sequences per
core, processed in lockstep by one SPMD program. GRU weights replicated.

Device layout notes:
  - Gates live on partitions ("transposed" layout): PSUM tile [128, 48]
    where cols = (gate_chunk m in 0..5) x (batch b in 0..7); partition p
    holds gate index m*128+p. This keeps every DVE/ACT op at free-dim
    8..32 (cheap) and avoids any per-step transpose of the hidden state.
  - x-side gate preactivations (gi = W_ih @ comp + biases) are precomputed
    for all steps with a dense matmul and injected into PSUM each step via
    an identity-weight matmul, so the in-loop critical path is only:
    W_hh matmuls -> sigmoid -> 2 DVE ops -> tanh -> 3 DVE ops -> cast.
"""

import os
import sys
import numpy as np
import ml_dtypes

B, T, D, H = 64, 1024, 256, 256
FLF, STEP_RED = 0.5, 0.2
NCORES = 8
BPC = B // NCORES  # batches per core
F32 = np.float32
BF16 = ml_dtypes.bfloat16

# dtype config for the scan matmuls (bf16 halves the LDWEIGHTS cost via FWL)
W_BF16 = True   # W_hh stationary + h rhs in bf16
GI_BF16 = True  # precomputed gi storage in bf16


# ----------------------------------------------------------------------------
# Host-side segmentation (mirrors the reference exactly; data-dependent)
# ----------------------------------------------------------------------------

def _segment_cm(x_np):
    B_, T_, D_ = x_np.shape
    enc = np.concatenate([np.zeros((1, D_), np.float32), x_np.reshape(-1, D_)], 0)
    cs1 = np.cumsum(enc, 0)
    cs2 = np.cumsum(enc * enc, 0)
    idx = np.arange(cs1.shape[0], dtype=np.int64)
    final_length = int(FLF * enc.shape[0])
    while idx.shape[0] > final_length:
        begs, ends = idx[:-2], idx[2:]
        s1 = cs1[ends] - cs1[begs]
        s2 = cs2[ends] - cs2[begs]
        n = (ends - begs).astype(np.float32)[:, None]
        var = (s2 / n - (s1 / n) ** 2).mean(1)
        diffs = np.sqrt(np.maximum(var, 0.0))
        diffs = np.concatenate([[1e10], diffs, [1e10]]).astype(np.float32)
        k = max(final_length, int(idx.shape[0] * STEP_RED))
        keep = np.sort(np.argsort(-diffs, kind='stable')[:k])
        idx = idx[keep]
    seq_end = np.arange(0, B_ * T_, T_, dtype=np.int64)
    idx = np.unique(np.concatenate([idx, seq_end]))
    cut = np.nonzero(idx % T_ == 0)[0]
    lens = np.diff(cut)
    rel = idx[1:] % T_
    groups = np.split(rel, np.cumsum(lens)[:-1])
    Lc = int(lens.max())
    seq = np.zeros((B_, Lc), np.int64)
    for b, g in enumerate(groups):
        seq[b, :len(g)] = g
    seq[seq == 0] = T_
    seq = np.concatenate([np.zeros((B_, 1), np.int64), seq], 1)
    fr = np.arange(T_)[None, None, :]
    cm = ((seq[:, :-1, None] <= fr) & (seq[:, 1:, None] > fr)).astype(np.float32)
    return cm  # [B, Lc, T]


# ----------------------------------------------------------------------------
# Bass program builder (one SPMD program; all shapes compile-time)
# ----------------------------------------------------------------------------

_PROGRAM_CACHE = {}


def _build_program(Lc, debug=False):
    import concourse.bacc as bacc
    import concourse.bass as bass
    import concourse.mybir as mybir
    import concourse.tile as tile
    from contextlib import ExitStack

    dt = mybir.dt
    f32 = dt.float32
    bf16 = dt.bfloat16
    AOP = mybir.AluOpType
    AF = mybir.ActivationFunctionType

    LCE = Lc + (Lc & 1)          # even width for 2x DVE mode on cm build
    LCP = ((Lc + 7) // 8) * 8    # gru dram rows padding (dma chunking)
    NCH = (Lc * BPC + 511) // 512  # N-chunks over (l, b) cols for gi matmul

    w_dt = bf16 if W_BF16 else f32
    gi_dt = bf16 if GI_BF16 else f32

    nc = bacc.Bacc("TRN2", target_bir_lowering=False, debug=False,
                   num_devices=NCORES)

    # ---- DRAM I/O ----
    x_d = nc.dram_tensor("x", [BPC, T // 128, 128, D], f32, kind="ExternalInput")
    wih_d = nc.dram_tensor("wih_t", [2, 128, 3 * H], f32, kind="ExternalInput")
    whh_d = nc.dram_tensor("whh_t", [2, 128, 3 * H], w_dt, kind="ExternalInput")
    ident_d = nc.dram_tensor("ident", [128, 128], gi_dt, kind="ExternalInput")
    bias_d = nc.dram_tensor("bias_comb", [128, 6], f32, kind="ExternalInput")
    bhhn_d = nc.dram_tensor("bhhn_rep", [128, 16], gi_dt, kind="ExternalInput")
    segp_d = nc.dram_tensor("segid_part", [128, BPC * (T // 128)], f32,
                            kind="ExternalInput")
    segi_d = nc.dram_tensor("seg_i16", [128, BPC * (T // 16)], dt.int16,
                            kind="ExternalInput")
    invn_d = nc.dram_tensor("invn", [1, BPC * LCE], f32, kind="ExternalInput")
    iota_d = nc.dram_tensor("iota_l", [128, LCE], f32, kind="ExternalInput")
    out_d = nc.dram_tensor("out", [BPC, T // 128, 128, H], f32,
                           kind="ExternalOutput")
    if debug:
        dbgc_d = nc.dram_tensor("dbg_comp", [2, 128, Lc, BPC], f32,
                                kind="ExternalOutput")
        dbgg_d = nc.dram_tensor("dbg_girz", [128, Lc, 32], gi_dt,
                                kind="ExternalOutput")
        dbgn_d = nc.dram_tensor("dbg_gin", [128, Lc, 16], gi_dt,
                                kind="ExternalOutput")
        dbgh_d = nc.dram_tensor("dbg_gru", [BPC, LCP, 2, 128], f32,
                                kind="ExternalOutput")

    NTC = T // 128  # 8 t-chunks

    with tile.TileContext(nc) as tc, ExitStack() as ctx:
        # ---------- persistent pools ----------
        const_pool = ctx.enter_context(tc.tile_pool(name="const", bufs=1))
        gi_pool = ctx.enter_context(tc.tile_pool(name="gi", bufs=1))
        gru_pool = ctx.enter_context(tc.tile_pool(name="gru", bufs=1))
        dram_pool = ctx.enter_context(tc.tile_pool(name="dram", bufs=1, space="DRAM"))

        wih_sb = [const_pool.tile([128, 3 * H], f32, tag=f"wih{k}",
                                  name=f"wih_sb{k}") for k in range(2)]
        whh_sb = [const_pool.tile([128, 3 * H], w_dt, tag=f"whh{k}",
                                  name=f"whh_sb{k}") for k in range(2)]
        for k in range(2):
            nc.sync.dma_start(wih_sb[k][:], wih_d[k])
            nc.sync.dma_start(whh_sb[k][:], whh_d[k])
        ident_sb = const_pool.tile([128, 128], gi_dt, tag="ident")
        nc.sync.dma_start(ident_sb[:], ident_d[:])
        bias_sb = const_pool.tile([128, 6], f32, tag="bias")
        nc.sync.dma_start(bias_sb[:], bias_d[:])
        bhhn_sb = const_pool.tile([128, 16], gi_dt, tag="bhhn")
        nc.sync.dma_start(bhhn_sb[:], bhhn_d[:])
        segp_sb = const_pool.tile([128, BPC * NTC], f32, tag="segp")
        nc.sync.dma_start(segp_sb[:], segp_d[:])
        iota_sb = const_pool.tile([128, LCE], f32, tag="iota")
        nc.sync.dma_start(iota_sb[:], iota_d[:])
        invn_sb = const_pool.tile([1, BPC * LCE], f32, tag="invn")
        nc.sync.dma_start(invn_sb[:], invn_d[:])
        ones_sb = const_pool.tile([1, 128], f32, tag="ones")
        nc.vector.memset(ones_sb[:], 1.0)
        zeros16 = const_pool.tile([128, 16], f32, tag="zeros16")
        nc.vector.memset(zeros16[:], 0.0)

        # gi storage (per step l: rz part [128, 32] cols (m,b) m<4;
        # n part [128, 16] cols (c,b))
        gi_rz = gi_pool.tile([128, Lc, 32], gi_dt, tag="girz")
        gi_n = gi_pool.tile([128, Lc, 16], gi_dt, tag="gin")

        # gru outputs, fp32, cols (l, c, b)
        gru_out = gru_pool.tile([128, Lc, 16], f32, tag="gruout")
        gdram = dram_pool.tile([BPC, LCP, 2, 128], f32, tag="gdram")

        # ---------- phase 1: compress + comp_int ----------
        comppool = ctx.enter_context(tc.tile_pool(name="compi", bufs=1))
        comp_int = [comppool.tile([128, Lc, BPC], f32, tag=f"comp{m}",
                                  name=f"comp_int{m}")
                    for m in range(2)]

        with tc.tile_pool(name="xp", bufs=10) as xpool, \
             tc.tile_pool(name="cmp", bufs=10) as cmpool, \
             tc.tile_pool(name="inv", bufs=2) as invpool, \
             tc.tile_pool(name="ps1", bufs=2, space="PSUM") as psum1:

            for b in range(BPC):
                # broadcast 1/n over partitions via K=1 matmul
                ps_inv = psum1.tile([128, LCE], f32, tag="psinv", bufs=1)
                o = b * LCE
                nc.tensor.matmul(ps_inv[:, 0:512], ones_sb[:],
                                 invn_sb[:, o:o + 512], start=True, stop=True)
                nc.tensor.matmul(ps_inv[:, 512:LCE], ones_sb[:],
                                 invn_sb[:, o + 512:o + LCE],
                                 start=True, stop=True)
                invbc = invpool.tile([128, LCE], f32, tag="invbc")
                nc.vector.tensor_copy(invbc[:], ps_inv[:])

                xt = []
                cmt = []
                for t in range(NTC):
                    x_tile = xpool.tile([128, D], f32, tag="xt")
                    nc.sync.dma_start(x_tile[:], x_d[b, t])
                    xt.append(x_tile)
                    cm_tile = cmpool.tile([128, LCE], f32, tag="cmt")
                    nc.vector.tensor_scalar(
                        cm_tile[:], iota_sb[:],
                        segp_sb[:, b * NTC + t: b * NTC + t + 1], None,
                        op0=AOP.is_equal)
                    cmt.append(cm_tile)

                for m in range(2):
                    ps_c = psum1.tile([128, LCE], f32, tag="psc")
                    for t in range(NTC):
                        nc.tensor.matmul(ps_c[:, 0:512],
                                         xt[t][:, m * 128:(m + 1) * 128],
                                         cmt[t][:, 0:512],
                                         start=(t == 0), stop=(t == NTC - 1))
                        nc.tensor.matmul(ps_c[:, 512:LCE],
                                         xt[t][:, m * 128:(m + 1) * 128],
                                         cmt[t][:, 512:LCE],
                                         start=(t == 0), stop=(t == NTC - 1))
                    # comp = raw_sum * (1/n), written at stride-8 cols (l,b)
                    nc.vector.scalar_tensor_tensor(
                        comp_int[m][:, :, b], ps_c[:, 0:Lc], 0.0,
                        invbc[:, 0:Lc], op0=AOP.bypass, op1=AOP.mult)

        # ---------- phase 2: gi = W_ih @ comp + bias ----------
        with tc.tile_pool(name="ps2", bufs=4, space="PSUM") as psum2:
            NLB = Lc * BPC
            for m in range(6):
                for nch in range(NCH):
                    j0 = nch * 512
                    j1 = min(j0 + 512, NLB)
                    # (l, b) cols j0..j1 <-> l in [j0/8, j1/8)
                    l0, l1 = j0 // BPC, j1 // BPC
                    nl = l1 - l0
                    ps_g = psum2.tile([128, 64, BPC], f32, tag="psg")
                    for k in range(2):
                        nc.tensor.matmul(
                            ps_g[:, 0:nl, :],
                            wih_sb[k][:, m * 128:(m + 1) * 128],
                            comp_int[k][:, l0:l1, :],
                            start=(k == 0), stop=(k == 1))
                    if m < 4:
                        dst = gi_rz[:, l0:l1, m * 8:(m + 1) * 8]
                    else:
                        dst = gi_n[:, l0:l1, (m - 4) * 8:(m - 3) * 8]
                    nc.vector.tensor_scalar(
                        dst, ps_g[:, 0:nl, :], bias_sb[:, m:m + 1], None,
                        op0=AOP.add)

        if debug:
            for m in range(2):
                nc.sync.dma_start(dbgc_d[m], comp_int[m][:])
            nc.sync.dma_start(dbgg_d[:], gi_rz[:])
            nc.sync.dma_start(dbgn_d[:], gi_n[:])

        # ---------- phase 3: GRU scan ----------
        with tc.tile_pool(name="scan", bufs=4) as spool, \
             tc.tile_pool(name="hb", bufs=3) as hpool, \
             tc.tile_pool(name="ps3", bufs=4, space="PSUM") as psum3:

            h_rhs = hpool.tile([128, 16], w_dt, tag="hbf")
            nc.vector.memset(h_rhs[:], 0.0)

            for l in range(Lc):
                # separate PSUM banks: each bank gets exactly ONE start=True
                # matmul (start clears has_written for the whole bank)
                ps_rz = psum3.tile([128, 32], f32, tag="pssrz")
                ps_n = psum3.tile([128, 16], f32, tag="pssn")
                # off-critical-path injection of x-side preactivations
                nc.tensor.matmul(ps_rz[:], ident_sb[:], gi_rz[:, l, :],
                                 start=True, stop=False)
                nc.tensor.matmul(ps_n[:], ident_sb[:], bhhn_sb[:],
                                 start=True, stop=False)
                # recurrent matmuls (rz chunks first: they gate the chain)
                for m in range(6):
                    ps = ps_rz if m < 4 else ps_n
                    j0 = m * 8 if m < 4 else (m - 4) * 8
                    for c in range(2):
                        nc.tensor.matmul(
                            ps[:, j0:j0 + 8],
                            whh_sb[c][:, m * 128:(m + 1) * 128],
                            h_rhs[:, c * 8:(c + 1) * 8],
                            start=False, stop=(c == 1))

                rz = spool.tile([128, 32], f32, tag="rz")
                nc.scalar.activation(rz[:], ps_rz[:], AF.Sigmoid)
                t1 = spool.tile([128, 16], f32, tag="t1")
                nc.vector.tensor_tensor(t1[:], rz[:, 0:16], ps_n[:],
                                        op=AOP.mult)
                t2 = spool.tile([128, 16], f32, tag="t2")
                nc.vector.tensor_tensor(t2[:], t1[:], gi_n[:, l, :], op=AOP.add)
                n_t = spool.tile([128, 16], f32, tag="nt")
                nc.scalar.activation(n_t[:], t2[:], AF.Tanh)

                h_prev = gru_out[:, l - 1, :] if l > 0 else zeros16[:]
                d_t = spool.tile([128, 16], f32, tag="dt")
                nc.vector.tensor_tensor(d_t[:], h_prev, n_t[:], op=AOP.subtract)
                e_t = spool.tile([128, 16], f32, tag="et")
                nc.vector.tensor_tensor(e_t[:], rz[:, 16:32], d_t[:], op=AOP.mult)
                nc.vector.tensor_tensor(gru_out[:, l, :], n_t[:], e_t[:],
                                        op=AOP.add)

                h_rhs = hpool.tile([128, 16], w_dt, tag="hbf")
                nc.vector.tensor_copy(h_rhs[:], gru_out[:, l, :])

                if l % 8 == 7 or l == Lc - 1:
                    l0 = (l // 8) * 8
                    for b in range(BPC):
                        dst = gdram[b, l0:l + 1, :, :].rearrange(
                            "l c p -> p l c")
                        src = gru_out[:, l0:l + 1, :].rearrange(
                            "p l (c b) -> p l c b", c=2)[:, :, :, b]
                        nc.sync.dma_start(dst, src)

        if debug:
            nc.sync.dma_start(dbgh_d[:], gdram[:])

        # ---------- phase 4: decompress via gather ----------
        with tc.tile_pool(name="gat", bufs=2) as gpool, \
             tc.tile_pool(name="gidx", bufs=1) as gidxpool:
            NI = T // 16
            segi_sb = gidxpool.tile([128, BPC * NI], mybir.dt.int16,
                                    tag="segi")
            nc.sync.dma_start(segi_sb[:], segi_d[:])
            for b in range(BPC):
                gout = gpool.tile([128, T // 128, H], f32, tag="gout")
                nc.gpsimd.dma_gather(
                    gout[:],
                    gdram[b].rearrange("l c p -> l (c p)"),
                    segi_sb[:, b * NI:(b + 1) * NI],
                    num_idxs=T, num_idxs_reg=T, elem_size=H)
                nc.sync.dma_start(out_d[b].rearrange("c p d -> p c d"), gout[:])

    nc.compile()
    return nc


def _get_program(Lc):
    if Lc not in _PROGRAM_CACHE:
        _PROGRAM_CACHE[Lc] = _build_program(Lc)
    return _PROGRAM_CACHE[Lc]


# ----------------------------------------------------------------------------
# Host-side prep + entry point
# ----------------------------------------------------------------------------

def _host_prep(x, W_ih, W_hh, b_ih, b_hh):
    x = np.asarray(x, np.float32)
    W_ih = np.asarray(W_ih, np.float32)
    W_hh = np.asarray(W_hh, np.float32)
    b_ih = np.asarray(b_ih, np.float32)
    b_hh = np.asarray(b_hh, np.float32)

    cm = _segment_cm(x)            # [B, Lc, T]
    Lc = cm.shape[1]
    LCE = Lc + (Lc & 1)
    segid = np.argmax(cm, axis=1).astype(np.float32)      # [B, T]
    lens = cm.sum(-1)                                     # [B, Lc]
    invn = (1.0 / np.maximum(lens, 1.0)).astype(np.float32)
    invn_p = np.zeros((B, LCE), np.float32)
    invn_p[:, :Lc] = invn

    w_np = BF16 if W_BF16 else np.float32
    gi_np = BF16 if GI_BF16 else np.float32

    wih_t = np.ascontiguousarray(W_ih.T.reshape(2, 128, 3 * H))
    whh_t = np.ascontiguousarray(W_hh.T.reshape(2, 128, 3 * H)).astype(w_np)
    ident = np.eye(128, dtype=gi_np)
    # bias folded into gi: r,z parts get b_ih+b_hh; n part gets b_ih only
    fold = np.concatenate([(b_ih + b_hh)[:2 * H], b_ih[2 * H:]])
    bias_comb = np.ascontiguousarray(fold.reshape(6, 128).T)
    # b_hh n-part, replicated over batch cols: col c*8+b -> b_hh[2H + c*128 + p]
    bhhn = b_hh[2 * H:].reshape(2, 128)
    bhhn_rep = np.repeat(bhhn[:, :, None], 8, axis=2)  # [c, p, b]
    bhhn_rep = np.ascontiguousarray(
        bhhn_rep.transpose(1, 0, 2).reshape(128, 16)).astype(gi_np)
    iota_l = np.broadcast_to(
        np.arange(LCE, dtype=np.float32)[None, :], (128, LCE)).copy()

    in_maps = []
    for c in range(NCORES):
        bs = slice(c * BPC, (c + 1) * BPC)
        xs = np.ascontiguousarray(x[bs].reshape(BPC, T // 128, 128, D))
        sp = np.ascontiguousarray(
            segid[bs].reshape(BPC, T // 128, 128).transpose(2, 0, 1)
            .reshape(128, BPC * (T // 128)))
        # gather indices. Empirically (probed on HW): for num_idxs=1024 the
        # SWDGE reads out[p, i] = in[wrap[p%16, i*8 + p//16]], so to get
        # out[p, i] = in[segid[i*128 + p]] we need
        # wrap[r, i*8+q] = segid[i*128 + q*16 + r].
        def _wrap_idx(s):
            w = s.astype(np.int16).reshape(8, 8, 16).transpose(2, 0, 1)
            return np.tile(w.reshape(16, 64), (8, 1))
        si = np.concatenate([_wrap_idx(segid[bs][b]) for b in range(BPC)],
                            axis=1)
        si = np.ascontiguousarray(si)
        in_maps.append({
            "x": xs,
            "wih_t": wih_t,
            "whh_t": whh_t,
            "ident": ident,
            "bias_comb": bias_comb,
            "bhhn_rep": bhhn_rep,
            "segid_part": sp,
            "seg_i16": si,
            "invn": np.ascontiguousarray(invn_p[bs].reshape(1, -1)),
            "iota_l": iota_l,
        })
    return Lc, in_maps


def kernel(x, W_ih, W_hh, b_ih, b_hh):
    from concourse import bass_utils

    Lc, in_maps = _host_prep(x, W_ih, W_hh, b_ih, b_hh)
    nc = _get_program(Lc)
    res = bass_utils.run_bass_kernel_spmd(nc, in_maps, list(range(NCORES)))
    outs = []
    for c in range(NCORES):
        o = res.results[c]["out"]  # [BPC, T//128, 128, H]
        outs.append(o.reshape(BPC, T, H))
    return np.concatenate(outs, 0).astype(np.float32)


# revision 22
# speedup vs baseline: 1.1289x; 1.1289x over previous
"""Trainium2 Bass kernel for nn_CPCAR_15960098472658 (ragged_sequence).

Pipeline (per batch element): variance-based segmentation (host, data
dependent) -> segment-mean compress (device matmul) -> GRU over compressed
sequence (device, serial scan) -> decompress by broadcasting each segment's
GRU output back to its frames (device DMA gather).

Sharding: data-parallel over batch. B=64 across 8 cores -> 8 sequences per
core, processed in lockstep by one SPMD program. GRU weights replicated.

Device layout notes:
  - Gates live on partitions ("transposed" layout): PSUM tile [128, 48]
    where cols = (gate_chunk m in 0..5) x (batch b in 0..7); partition p
    holds gate index m*128+p. This keeps every DVE/ACT op at free-dim
    8..32 (cheap) and avoids any per-step transpose of the hidden state.
  - x-side gate preactivations (gi = W_ih @ comp + biases) are precomputed
    for all steps with a dense matmul and injected into PSUM each step via
    an identity-weight matmul, so the in-loop critical path is only:
    W_hh matmuls -> sigmoid -> 2 DVE ops -> tanh -> 3 DVE ops -> cast.
"""

import os
import sys
import numpy as np
import ml_dtypes

B, T, D, H = 64, 1024, 256, 256
FLF, STEP_RED = 0.5, 0.2
NCORES = 8
BPC = B // NCORES  # batches per core
F32 = np.float32
BF16 = ml_dtypes.bfloat16

# dtype config for the scan matmuls (bf16 halves the LDWEIGHTS cost via FWL)
W_BF16 = True   # W_hh stationary + h rhs in bf16
GI_BF16 = True  # precomputed gi storage in bf16


# ----------------------------------------------------------------------------
# Host-side segmentation (mirrors the reference exactly; data-dependent)
# ----------------------------------------------------------------------------

def _segment_cm(x_np):
    B_, T_, D_ = x_np.shape
    enc = np.concatenate([np.zeros((1, D_), np.float32), x_np.reshape(-1, D_)], 0)
    cs1 = np.cumsum(enc, 0)
    cs2 = np.cumsum(enc * enc, 0)
    idx = np.arange(cs1.shape[0], dtype=np.int64)
    final_length = int(FLF * enc.shape[0])
    while idx.shape[0] > final_length:
        begs, ends = idx[:-2], idx[2:]
        s1 = cs1[ends] - cs1[begs]
        s2 = cs2[ends] - cs2[begs]
        n = (ends - begs).astype(np.float32)[:, None]
        var = (s2 / n - (s1 / n) ** 2).mean(1)
        diffs = np.sqrt(np.maximum(var, 0.0))
        diffs = np.concatenate([[1e10], diffs, [1e10]]).astype(np.float32)
        k = max(final_length, int(idx.shape[0] * STEP_RED))
        keep = np.sort(np.argsort(-diffs, kind='stable')[:k])
        idx = idx[keep]
    seq_end = np.arange(0, B_ * T_, T_, dtype=np.int64)
    idx = np.unique(np.concatenate([idx, seq_end]))
    cut = np.nonzero(idx % T_ == 0)[0]
    lens = np.diff(cut)
    rel = idx[1:] % T_
    groups = np.split(rel, np.cumsum(lens)[:-1])
    Lc = int(lens.max())
    seq = np.zeros((B_, Lc), np.int64)
    for b, g in enumerate(groups):
        seq[b, :len(g)] = g
    seq[seq == 0] = T_
    seq = np.concatenate([np.zeros((B_, 1), np.int64), seq], 1)
    fr = np.arange(T_)[None, None, :]
    cm = ((seq[:, :-1, None] <= fr) & (seq[:, 1:, None] > fr)).astype(np.float32)
    return cm  # [B, Lc, T]


# ----------------------------------------------------------------------------
# Bass program builder (one SPMD program; all shapes compile-time)
# ----------------------------------------------------------------------------

_PROGRAM_CACHE = {}


def _build_program(Lc, debug=False):
    import concourse.bacc as bacc
    import concourse.bass as bass
    import concourse.mybir as mybir
    import concourse.tile as tile
    from contextlib import ExitStack

    dt = mybir.dt
    f32 = dt.float32
    bf16 = dt.bfloat16
    AOP = mybir.AluOpType
    AF = mybir.ActivationFunctionType

    LCE = Lc + (Lc & 1)          # even width for 2x DVE mode on cm build
    LCP = ((Lc + 7) // 8) * 8    # gru dram rows padding (dma chunking)
    NCH = (Lc * BPC + 511) // 512  # N-chunks over (l, b) cols for gi matmul

    w_dt = bf16 if W_BF16 else f32
    gi_dt = bf16 if GI_BF16 else f32

    nc = bacc.Bacc("TRN2", target_bir_lowering=False, debug=False,
                   num_devices=NCORES)

    # ---- DRAM I/O ----
    x_d = nc.dram_tensor("x", [BPC, T // 128, 128, D], f32, kind="ExternalInput")
    wih_d = nc.dram_tensor("wih_t", [2, 128, 3 * H], f32, kind="ExternalInput")
    whh_d = nc.dram_tensor("whh_t", [2, 128, 3 * H], w_dt, kind="ExternalInput")
    ident_d = nc.dram_tensor("ident", [128, 128], gi_dt, kind="ExternalInput")
    bias_d = nc.dram_tensor("bias_comb", [128, 6], f32, kind="ExternalInput")
    bhhn_d = nc.dram_tensor("bhhn_rep", [128, 16], gi_dt, kind="ExternalInput")
    segp_d = nc.dram_tensor("segid_part", [128, BPC * (T // 128)], f32,
                            kind="ExternalInput")
    segi_d = nc.dram_tensor("seg_i16", [128, BPC * (T // 16)], dt.int16,
                            kind="ExternalInput")
    invn_d = nc.dram_tensor("invn", [1, BPC * LCE], f32, kind="ExternalInput")
    iota_d = nc.dram_tensor("iota_l", [128, LCE], f32, kind="ExternalInput")
    out_d = nc.dram_tensor("out", [BPC, T // 128, 128, H], f32,
                           kind="ExternalOutput")
    if debug:
        dbgc_d = nc.dram_tensor("dbg_comp", [2, 128, Lc, BPC], f32,
                                kind="ExternalOutput")
        dbgg_d = nc.dram_tensor("dbg_girz", [128, Lc, 32], gi_dt,
                                kind="ExternalOutput")
        dbgn_d = nc.dram_tensor("dbg_gin", [128, Lc, 16], gi_dt,
                                kind="ExternalOutput")
        dbgh_d = nc.dram_tensor("dbg_gru", [BPC, LCP, 2, 128], f32,
                                kind="ExternalOutput")

    NTC = T // 128  # 8 t-chunks

    with tile.TileContext(nc) as tc, ExitStack() as ctx:
        # ---------- persistent pools ----------
        const_pool = ctx.enter_context(tc.tile_pool(name="const", bufs=1))
        gi_pool = ctx.enter_context(tc.tile_pool(name="gi", bufs=1))
        gru_pool = ctx.enter_context(tc.tile_pool(name="gru", bufs=1))
        dram_pool = ctx.enter_context(tc.tile_pool(name="dram", bufs=1, space="DRAM"))

        wih_sb = [const_pool.tile([128, 3 * H], f32, tag=f"wih{k}",
                                  name=f"wih_sb{k}") for k in range(2)]
        whh_sb = [const_pool.tile([128, 3 * H], w_dt, tag=f"whh{k}",
                                  name=f"whh_sb{k}") for k in range(2)]
        for k in range(2):
            nc.sync.dma_start(wih_sb[k][:], wih_d[k])
            nc.sync.dma_start(whh_sb[k][:], whh_d[k])
        ident_sb = const_pool.tile([128, 128], gi_dt, tag="ident")
        nc.sync.dma_start(ident_sb[:], ident_d[:])
        bias_sb = const_pool.tile([128, 6], f32, tag="bias")
        nc.sync.dma_start(bias_sb[:], bias_d[:])
        bhhn_sb = const_pool.tile([128, 16], gi_dt, tag="bhhn")
        nc.sync.dma_start(bhhn_sb[:], bhhn_d[:])
        segp_sb = const_pool.tile([128, BPC * NTC], f32, tag="segp")
        nc.sync.dma_start(segp_sb[:], segp_d[:])
        iota_sb = const_pool.tile([128, LCE], f32, tag="iota")
        nc.sync.dma_start(iota_sb[:], iota_d[:])
        invn_sb = const_pool.tile([1, BPC * LCE], f32, tag="invn")
        nc.sync.dma_start(invn_sb[:], invn_d[:])
        ones_sb = const_pool.tile([1, 128], f32, tag="ones")
        nc.vector.memset(ones_sb[:], 1.0)
        zeros16 = const_pool.tile([128, 16], f32, tag="zeros16")
        nc.vector.memset(zeros16[:], 0.0)

        # gi storage (per step l: rz part [128, 32] cols (m,b) m<4;
        # n part [128, 16] cols (c,b))
        gi_rz = gi_pool.tile([128, Lc, 32], gi_dt, tag="girz")
        gi_n = gi_pool.tile([128, Lc, 16], gi_dt, tag="gin")

        # gru outputs, fp32, cols (l, c, b)
        gru_out = gru_pool.tile([128, Lc, 16], f32, tag="gruout")
        gdram = dram_pool.tile([BPC, LCP, 2, 128], f32, tag="gdram")

        # ---------- phase 1: compress + comp_int ----------
        comppool = ctx.enter_context(tc.tile_pool(name="compi", bufs=1))
        comp_int = [comppool.tile([128, Lc, BPC], f32, tag=f"comp{m}",
                                  name=f"comp_int{m}")
                    for m in range(2)]

        with tc.tile_pool(name="xp", bufs=10) as xpool, \
             tc.tile_pool(name="cmp", bufs=10) as cmpool, \
             tc.tile_pool(name="inv", bufs=2) as invpool, \
             tc.tile_pool(name="ps1", bufs=2, space="PSUM") as psum1:

            for b in range(BPC):
                # broadcast 1/n over partitions via K=1 matmul
                ps_inv = psum1.tile([128, LCE], f32, tag="psinv", bufs=1)
                o = b * LCE
                nc.tensor.matmul(ps_inv[:, 0:512], ones_sb[:],
                                 invn_sb[:, o:o + 512], start=True, stop=True)
                nc.tensor.matmul(ps_inv[:, 512:LCE], ones_sb[:],
                                 invn_sb[:, o + 512:o + LCE],
                                 start=True, stop=True)
                invbc = invpool.tile([128, LCE], f32, tag="invbc")
                nc.vector.tensor_copy(invbc[:], ps_inv[:])

                xt = []
                cmt = []
                for t in range(NTC):
                    x_tile = xpool.tile([128, D], f32, tag="xt")
                    nc.sync.dma_start(x_tile[:], x_d[b, t])
                    xt.append(x_tile)
                    cm_tile = cmpool.tile([128, LCE], f32, tag="cmt")
                    nc.vector.tensor_scalar(
                        cm_tile[:], iota_sb[:],
                        segp_sb[:, b * NTC + t: b * NTC + t + 1], None,
                        op0=AOP.is_equal)
                    cmt.append(cm_tile)

                for m in range(2):
                    ps_c = psum1.tile([128, LCE], f32, tag="psc")
                    for t in range(NTC):
                        nc.tensor.matmul(ps_c[:, 0:512],
                                         xt[t][:, m * 128:(m + 1) * 128],
                                         cmt[t][:, 0:512],
                                         start=(t == 0), stop=(t == NTC - 1))
                        nc.tensor.matmul(ps_c[:, 512:LCE],
                                         xt[t][:, m * 128:(m + 1) * 128],
                                         cmt[t][:, 512:LCE],
                                         start=(t == 0), stop=(t == NTC - 1))
                    # comp = raw_sum * (1/n), written at stride-8 cols (l,b)
                    nc.vector.scalar_tensor_tensor(
                        comp_int[m][:, :, b], ps_c[:, 0:Lc], 0.0,
                        invbc[:, 0:Lc], op0=AOP.bypass, op1=AOP.mult)

        # ---------- phase 2: gi = W_ih @ comp + bias ----------
        with tc.tile_pool(name="ps2", bufs=4, space="PSUM") as psum2:
            NLB = Lc * BPC
            for m in range(6):
                for nch in range(NCH):
                    j0 = nch * 512
                    j1 = min(j0 + 512, NLB)
                    # (l, b) cols j0..j1 <-> l in [j0/8, j1/8)
                    l0, l1 = j0 // BPC, j1 // BPC
                    nl = l1 - l0
                    ps_g = psum2.tile([128, 64, BPC], f32, tag="psg")
                    for k in range(2):
                        nc.tensor.matmul(
                            ps_g[:, 0:nl, :],
                            wih_sb[k][:, m * 128:(m + 1) * 128],
                            comp_int[k][:, l0:l1, :],
                            start=(k == 0), stop=(k == 1))
                    if m < 4:
                        dst = gi_rz[:, l0:l1, m * 8:(m + 1) * 8]
                    else:
                        dst = gi_n[:, l0:l1, (m - 4) * 8:(m - 3) * 8]
                    nc.vector.tensor_scalar(
                        dst, ps_g[:, 0:nl, :], bias_sb[:, m:m + 1], None,
                        op0=AOP.add)

        if debug:
            for m in range(2):
                nc.sync.dma_start(dbgc_d[m], comp_int[m][:])
            nc.sync.dma_start(dbgg_d[:], gi_rz[:])
            nc.sync.dma_start(dbgn_d[:], gi_n[:])

        # ---------- phase 3: GRU scan ----------
        with tc.tile_pool(name="scan", bufs=4) as spool, \
             tc.tile_pool(name="hb", bufs=3) as hpool, \
             tc.tile_pool(name="ps3", bufs=4, space="PSUM") as psum3:

            h_rhs = hpool.tile([128, 16], w_dt, tag="hbf")
            nc.vector.memset(h_rhs[:], 0.0)

            for l in range(Lc):
                # separate PSUM banks: each bank gets exactly ONE start=True
                # matmul (start clears has_written for the whole bank)
                ps_rz = psum3.tile([128, 32], f32, tag="pssrz")
                ps_n = psum3.tile([128, 16], f32, tag="pssn")
                # off-critical-path injection of x-side preactivations
                nc.tensor.matmul(ps_rz[:], ident_sb[:], gi_rz[:, l, :],
                                 start=True, stop=False)
                nc.tensor.matmul(ps_n[:], ident_sb[:], bhhn_sb[:],
                                 start=True, stop=False)
                # recurrent matmuls (rz chunks first: they gate the chain)
                for m in range(6):
                    ps = ps_rz if m < 4 else ps_n
                    j0 = m * 8 if m < 4 else (m - 4) * 8
                    for c in range(2):
                        nc.tensor.matmul(
                            ps[:, j0:j0 + 8],
                            whh_sb[c][:, m * 128:(m + 1) * 128],
                            h_rhs[:, c * 8:(c + 1) * 8],
                            start=False, stop=(c == 1))

                # z-gate weights/biases are negated on host, so sigmoid
                # yields w = 1 - z directly:  h' = w*n + (h - w*h)
                rz = spool.tile([128, 32], f32, tag="rz")
                nc.scalar.activation(rz[:], ps_rz[:], AF.Sigmoid)
                t1 = spool.tile([128, 16], f32, tag="t1")
                nc.vector.tensor_tensor(t1[:], rz[:, 0:16], ps_n[:],
                                        op=AOP.mult)
                t2 = spool.tile([128, 16], f32, tag="t2")
                nc.vector.tensor_tensor(t2[:], t1[:], gi_n[:, l, :], op=AOP.add)
                n_t = spool.tile([128, 16], f32, tag="nt")
                nc.scalar.activation(n_t[:], t2[:], AF.Tanh)

                h_prev = gru_out[:, l - 1, :] if l > 0 else zeros16[:]
                # off-critical-path (runs while ACT evaluates tanh):
                f_t = spool.tile([128, 16], f32, tag="ft")
                nc.vector.tensor_tensor(f_t[:], rz[:, 16:32], h_prev,
                                        op=AOP.mult)          # w*h
                u_t = spool.tile([128, 16], f32, tag="ut")
                nc.vector.tensor_tensor(u_t[:], h_prev, f_t[:],
                                        op=AOP.subtract)      # h - w*h
                # critical path after tanh: v = w*n ; h' = v + u
                v_t = spool.tile([128, 16], f32, tag="vt")
                nc.vector.tensor_tensor(v_t[:], rz[:, 16:32], n_t[:],
                                        op=AOP.mult)
                h_rhs = hpool.tile([128, 16], w_dt, tag="hbf")
                nc.vector.tensor_tensor(h_rhs[:], v_t[:], u_t[:], op=AOP.add)
                # off-chain fp32 copy for the gru output buffer
                nc.vector.tensor_tensor(gru_out[:, l, :], v_t[:], u_t[:],
                                        op=AOP.add)

                if l % 8 == 7 or l == Lc - 1:
                    l0 = (l // 8) * 8
                    for b in range(BPC):
                        dst = gdram[b, l0:l + 1, :, :].rearrange(
                            "l c p -> p l c")
                        src = gru_out[:, l0:l + 1, :].rearrange(
                            "p l (c b) -> p l c b", c=2)[:, :, :, b]
                        nc.sync.dma_start(dst, src)

        if debug:
            nc.sync.dma_start(dbgh_d[:], gdram[:])

        # ---------- phase 4: decompress via gather ----------
        with tc.tile_pool(name="gat", bufs=2) as gpool, \
             tc.tile_pool(name="gidx", bufs=1) as gidxpool:
            NI = T // 16
            segi_sb = gidxpool.tile([128, BPC * NI], mybir.dt.int16,
                                    tag="segi")
            nc.sync.dma_start(segi_sb[:], segi_d[:])
            for b in range(BPC):
                gout = gpool.tile([128, T // 128, H], f32, tag="gout")
                nc.gpsimd.dma_gather(
                    gout[:],
                    gdram[b].rearrange("l c p -> l (c p)"),
                    segi_sb[:, b * NI:(b + 1) * NI],
                    num_idxs=T, num_idxs_reg=T, elem_size=H)
                nc.sync.dma_start(out_d[b].rearrange("c p d -> p c d"), gout[:])

    nc.compile()
    return nc


def _get_program(Lc):
    if Lc not in _PROGRAM_CACHE:
        _PROGRAM_CACHE[Lc] = _build_program(Lc)
    return _PROGRAM_CACHE[Lc]


# ----------------------------------------------------------------------------
# Host-side prep + entry point
# ----------------------------------------------------------------------------

def _host_prep(x, W_ih, W_hh, b_ih, b_hh):
    x = np.asarray(x, np.float32)
    W_ih = np.asarray(W_ih, np.float32)
    W_hh = np.asarray(W_hh, np.float32)
    b_ih = np.asarray(b_ih, np.float32)
    b_hh = np.asarray(b_hh, np.float32)

    cm = _segment_cm(x)            # [B, Lc, T]
    Lc = cm.shape[1]
    LCE = Lc + (Lc & 1)
    segid = np.argmax(cm, axis=1).astype(np.float32)      # [B, T]
    lens = cm.sum(-1)                                     # [B, Lc]
    invn = (1.0 / np.maximum(lens, 1.0)).astype(np.float32)
    invn_p = np.zeros((B, LCE), np.float32)
    invn_p[:, :Lc] = invn

    w_np = BF16 if W_BF16 else np.float32
    gi_np = BF16 if GI_BF16 else np.float32

    # negate the z-gate rows so sigmoid yields w = 1 - z on device
    zneg = np.ones((3 * H,), np.float32)
    zneg[H:2 * H] = -1.0
    wih_t = np.ascontiguousarray((W_ih.T * zneg).reshape(2, 128, 3 * H))
    whh_t = np.ascontiguousarray(
        (W_hh.T * zneg).reshape(2, 128, 3 * H)).astype(w_np)
    ident = np.eye(128, dtype=gi_np)
    # bias folded into gi: r,z parts get b_ih+b_hh; n part gets b_ih only
    fold = np.concatenate([(b_ih + b_hh)[:2 * H], b_ih[2 * H:]]) * zneg
    bias_comb = np.ascontiguousarray(fold.reshape(6, 128).T)
    # b_hh n-part, replicated over batch cols: col c*8+b -> b_hh[2H + c*128 + p]
    bhhn = b_hh[2 * H:].reshape(2, 128)
    bhhn_rep = np.repeat(bhhn[:, :, None], 8, axis=2)  # [c, p, b]
    bhhn_rep = np.ascontiguousarray(
        bhhn_rep.transpose(1, 0, 2).reshape(128, 16)).astype(gi_np)
    iota_l = np.broadcast_to(
        np.arange(LCE, dtype=np.float32)[None, :], (128, LCE)).copy()

    in_maps = []
    for c in range(NCORES):
        bs = slice(c * BPC, (c + 1) * BPC)
        xs = np.ascontiguousarray(x[bs].reshape(BPC, T // 128, 128, D))
        sp = np.ascontiguousarray(
            segid[bs].reshape(BPC, T // 128, 128).transpose(2, 0, 1)
            .reshape(128, BPC * (T // 128)))
        # gather indices. Empirically (probed on HW): for num_idxs=1024 the
        # SWDGE reads out[p, i] = in[wrap[p%16, i*8 + p//16]], so to get
        # out[p, i] = in[segid[i*128 + p]] we need
        # wrap[r, i*8+q] = segid[i*128 + q*16 + r].
        def _wrap_idx(s):
            w = s.astype(np.int16).reshape(8, 8, 16).transpose(2, 0, 1)
            return np.tile(w.reshape(16, 64), (8, 1))
        si = np.concatenate([_wrap_idx(segid[bs][b]) for b in range(BPC)],
                            axis=1)
        si = np.ascontiguousarray(si)
        in_maps.append({
            "x": xs,
            "wih_t": wih_t,
            "whh_t": whh_t,
            "ident": ident,
            "bias_comb": bias_comb,
            "bhhn_rep": bhhn_rep,
            "segid_part": sp,
            "seg_i16": si,
            "invn": np.ascontiguousarray(invn_p[bs].reshape(1, -1)),
            "iota_l": iota_l,
        })
    return Lc, in_maps


def kernel(x, W_ih, W_hh, b_ih, b_hh):
    from concourse import bass_utils

    Lc, in_maps = _host_prep(x, W_ih, W_hh, b_ih, b_hh)
    nc = _get_program(Lc)
    res = bass_utils.run_bass_kernel_spmd(nc, in_maps, list(range(NCORES)))
    outs = []
    for c in range(NCORES):
        o = res.results[c]["out"]  # [BPC, T//128, 128, H]
        outs.append(o.reshape(BPC, T, H))
    return np.concatenate(outs, 0).astype(np.float32)
